# revision 22
# baseline (speedup 1.0000x reference)
"""Trainium2 Bass kernel for single-head causal attention.

Problem: x[4, 2048, 1024] fp32; wq/wk/wv [1024, 1024] (torch layout [d_out, d_in]).
  q = x @ wq.T ; k = x @ wk.T ; v = x @ wv.T  (per batch)
  out = softmax(causal(q @ k.T) / 32) @ v

Sharding (8 cores): core c = 2*b + h owns batch b and half of its query rows.
Query rows are split between the two cores of a batch by parity *within* each
1024-row group so both cores see an identical causal work profile -> the SPMD
program is fully uniform; only data (inputs) differ per core. Cores are fully
independent (no collectives / cross-core traffic).

Key algebraic folding (removes the K and V projections entirely):
  scores = (X Wq^T)(X Wk^T)^T = X (Wq^T Wk) X^T
         = U X^T   with U := Xq A,  A := Wq^T Wk  (precomputed on HOST)
  out    = P V / l = (P X) Wv^T / l = T Wv^T / l  with T := P X
So the device only computes:  U (one [1024x1024x1024] projection per core),
scores against raw X^T, exp, T = P X, and out = T Wv^T. Per-core PE work is
~283K PE-columns vs ~490K for the direct form.

All matmuls are bf16 (1 cycle/row PE rate at any N, LDWEIGHTS hidden, half
the DMA bytes of fp32); PSUM accumulation stays fp32.

Per-core device program:
  phase 1: uT[di, q'] = A-chunks.T @ xqT   (SBUF-resident, like a projection)
           xt[dc] (= X^T, scores lhsT) and xr[kb] (= X rows, T lhsT) stream
           in by plain DMA (inputs, nothing to compute).
  phase 2: per group g (2 groups of 512 own-q columns, kb = 8g+8 key blocks):
      scoresT[k, q'] = sum_d xt.T @ uT  (PSUM, 8 d-chunk matmuls, exact
      causal width per key block; stale PSUM left of the band is killed by
      the additive mask); additive causal mask (DVE, bf16 -1e9 tiles);
      p = exp(scores/32) (ACT, PSUM->SBUF bf16); l[q'] += ones.T @ p (PE);
      T:  tT[d, q'] += xr-slices.T @ p   (per d-chunk, exact causal width)
      out[q', o] = sum_d tT-slices.T @ wvT-chunks, then ACT copy divides by
      l via per-partition scale AP (l transposed into lanes by 4 tiny column
      DMAs, then reciprocal on [128, 4]).
"""

import os
import sys
import types
from contextlib import ExitStack

for _p in ("/opt/trn_rl_repo", "/root/.axon_site/_ro/trn_rl_repo"):
    if os.path.isdir(_p) and _p not in sys.path:
        sys.path.insert(0, _p)

import numpy as np

import concourse.bacc as bacc
import concourse.mybir as mybir
import concourse.tile as tile
from concourse.bass_utils import run_bass_kernel_spmd

F32 = mybir.dt.float32
BF16 = mybir.dt.bfloat16

B, S, D = 4, 2048, 1024
P = 128
DC = D // P      # 8 d-chunks
SKB = S // P     # 16 key blocks
G = 2            # query groups per core
QW = 512         # query columns per group per core
SQ = G * QW      # 1024 own query rows per core
N_CORES = 8
SCALE = 1.0 / 32.0  # 1/sqrt(D)
NEG = -1e9


def _install_axon_profile_hook():
    """Provide antenv.axon_hooks (absent in this image) so trace=True works."""
    name = "antenv.axon_hooks"
    if name in sys.modules:
        return
    mod = types.ModuleType(name)
    _hook = [None]
    mod.set_axon_ntff_profile_hook = lambda h: _hook.__setitem__(0, h)
    mod.get_axon_ntff_profile_hook = lambda: _hook[0]
    sys.modules[name] = mod
    try:
        import antenv

        antenv.axon_hooks = mod
        from trn_agent_boot.trn_boot import _ntff_profile_via_ctypes

        mod.set_axon_ntff_profile_hook(
            _ntff_profile_via_ctypes("/opt/axon/libaxon_pjrt.so")
        )
    except Exception:
        pass


def _build_program():
    nc = bacc.Bacc("TRN2", target_bir_lowering=False, debug=False,
                   num_devices=N_CORES)

    xt = nc.dram_tensor("xt", [D, S], BF16, kind="ExternalInput").ap()
    xr = nc.dram_tensor("xr", [S, D], BF16, kind="ExternalInput").ap()
    # xq/at/wvt come pre-arranged on the host into exact SBUF tile layout
    # so every load is a contiguous 128x4KB DMA (128 descriptors), not a
    # strided rearrange (1024 small descriptors, ~10x slower issue).
    xq = nc.dram_tensor("xq", [2 * P, DC * 512], BF16,
                        kind="ExternalInput").ap()
    at = nc.dram_tensor("at", [4 * P, DC * 256], BF16,
                        kind="ExternalInput").ap()
    wvt = nc.dram_tensor("wvt", [2 * P, DC * 512], BF16,
                         kind="ExternalInput").ap()
    ones_in = nc.dram_tensor("ones", [P, 512], BF16, kind="ExternalInput").ap()
    mask = nc.dram_tensor("mask", [P, 8 * 64], BF16, kind="ExternalInput").ap()
    out = nc.dram_tensor("out", [SQ, D], F32, kind="ExternalOutput").ap()

    with tile.TileContext(nc, pool_alloc_mode="queue") as tc, ExitStack() as es:
        const = es.enter_context(tc.tile_pool(name="const", bufs=1))
        utpool = es.enter_context(tc.tile_pool(name="utpool", bufs=8))
        xtpool = es.enter_context(tc.tile_pool(name="xtpool", bufs=8))
        xrpool = es.enter_context(tc.tile_pool(name="xrpool", bufs=16))
        wvpool = es.enter_context(tc.tile_pool(name="wvpool", bufs=2))
        maskpool = es.enter_context(tc.tile_pool(name="maskpool", bufs=1))

        ones1 = const.tile([P, 512], BF16)
        nc.sync.dma_start(out=ones1[:], in_=ones_in[:])

        p1 = ExitStack()
        # A o-quarter tiles [128, 8*256]: quarter i holds di-cols
        # [256i, 256(i+1)) for all 8 dj-chunks (lhsT slices for U).
        apool = p1.enter_context(tc.tile_pool(name="apool", bufs=4))
        xqpool = p1.enter_context(tc.tile_pool(name="xqpool", bufs=2))
        pp = p1.enter_context(tc.tile_pool(name="pp", bufs=2, space="PSUM"))

        def load_a(i, eng=None):
            w = apool.tile([P, DC * 256], BF16, tag="a")
            (eng or nc.sync).dma_start(
                out=w[:], in_=at[i * P:(i + 1) * P, :])
            return w

        def a_lhsT(a_sb, oc, dc):
            base = dc * 256 + (oc % 2) * P
            return a_sb[oc // 2][:, base:base + P]

        def load_xq(st, eng=None):
            t = xqpool.tile([P, DC * 512], BF16, tag="xq")
            (eng or nc.sync).dma_start(
                out=t[:], in_=xq[st * P:(st + 1) * P, :])
            return t

        # PE warmup: throwaway matmuls on the ones tile (lands in ~1us)
        # keep the PE busy through its p-state ramp while inputs stream in
        wu_ps = p1.enter_context(tc.tile_pool(name="wups", bufs=1,
                                              space="PSUM"))
        wups = wu_ps.tile([P, 512], F32, tag="wu")
        for i in range(10):
            nc.tensor.matmul(
                wups[:], ones1[:, 0:P], ones1[:, 0:512],
                start=(i == 0), stop=(i == 9), skip_group_check=True)

        # startup: first A quarter + first xq tile unblock the first U
        # accumulation group asap. Critical loads stay on the two fast HWDGE
        # queues (sync/scalar); the slow gpsimd SWDGE queue only carries bulk
        # tiles that aren't needed until scores/T time.
        a_sb = [load_a(0, eng=nc.sync)]
        xq_sb = [load_xq(0, eng=nc.scalar)]
        a_sb += [load_a(1, eng=nc.sync), load_a(2, eng=nc.scalar),
                 load_a(3, eng=nc.sync)]
        xq_sb.append(load_xq(1, eng=nc.scalar))

        mask_sb = maskpool.tile([P, 8 * 64], BF16, tag="mask")
        nc.scalar.dma_start(out=mask_sb[:], in_=mask[:])

        # resident X^T chunk tiles (scores lhsT): one contiguous DMA each,
        # spread over all three queues (needed from scores-g0 time on)
        xt_engs = [nc.gpsimd, nc.gpsimd, nc.gpsimd, nc.gpsimd,
                   nc.sync, nc.sync, nc.scalar, nc.scalar]
        xt_sb = []
        for dc in range(DC):
            t = xtpool.tile([P, S], BF16, tag="xt", name=f"xt{dc}")
            xt_engs[dc].dma_start(out=t[:], in_=xt[dc * P:(dc + 1) * P, :])
            xt_sb.append(t)

        # resident X row-block tiles (T lhsT): one contiguous DMA each
        xr_engs = [nc.gpsimd] * 8 + [nc.sync] * 4 + [nc.scalar] * 4
        xr_sb = []
        for kb in range(SKB):
            t = xrpool.tile([P, D], BF16, tag="xr", name=f"xr{kb}")
            xr_engs[kb % 8 if kb < 8 else kb].dma_start(
                out=t[:], in_=xr[kb * P:(kb + 1) * P, :])
            xr_sb.append(t)

        # wv^T o-half tiles [128, 8*512] (moving operand for out = T Wv^T)
        def load_wv(i, eng=None):
            w = wvpool.tile([P, DC * 512], BF16, tag="wv")
            (eng or nc.sync).dma_start(
                out=w[:], in_=wvt[i * P:(i + 1) * P, :])
            return w

        wv_sb = [load_wv(0, eng=nc.sync), load_wv(1, eng=nc.scalar)]

        # ---- phase 1: uT projection (resident) ----
        ut = [utpool.tile([P, SQ], BF16, tag="ut", name=f"ut{i}")
              for i in range(DC)]
        for st in range(2):
            for oc in range(DC):
                ps = pp.tile([P, 512], F32, tag="pp")
                for dc in range(DC):
                    nc.tensor.matmul(
                        ps[:],
                        a_lhsT(a_sb, oc, dc),
                        xq_sb[st][:, dc * 512:(dc + 1) * 512],
                        start=(dc == 0), stop=(dc == DC - 1),
                    )
                nc.vector.tensor_copy(
                    ut[oc][:, st * 512:(st + 1) * 512], ps[:])
        p1.close()

        # ---- phase 2: attention ----
        ptpool = es.enter_context(tc.tile_pool(name="ptpool", bufs=16))
        ttpool = es.enter_context(tc.tile_pool(name="ttpool", bufs=10))
        linvtpool = es.enter_context(tc.tile_pool(name="linvtpool", bufs=2))
        outpool = es.enter_context(tc.tile_pool(name="outpool", bufs=2))
        ps_s = es.enter_context(tc.tile_pool(name="ps_s", bufs=2, space="PSUM"))
        ps_l = es.enter_context(tc.tile_pool(name="ps_l", bufs=1, space="PSUM"))
        ps_t = es.enter_context(tc.tile_pool(name="ps_t", bufs=2, space="PSUM"))
        ps_o = es.enter_context(tc.tile_pool(name="ps_o", bufs=2, space="PSUM"))

        # Per-group state; group g has U = 8g+8 key-block units.
        l_ps = {}
        pts = {0: [None] * 8, 1: [None] * 16}
        score_ps = {0: [None] * 8, 1: [None] * 16}
        linv_t = {}
        tt = {}

        def rs_of(g, j):
            # key block j contributes only to q' >= 64*(j-8g); exact causal
            # width (bf16 runs full rate at any N). Everything downstream
            # (exp, wedge, l, T) reads only [rs:], so PSUM left of rs is
            # never touched.
            jj = j - 8 * g
            return 64 * jj if j >= 8 * g else 0

        def emit_scores(g, j):
            rs = rs_of(g, j)
            ps = ps_s.tile([P, QW], F32, tag="s")
            for dc in range(DC):
                nc.tensor.matmul(
                    ps[:, rs:],
                    xt_sb[dc][:, j * P:(j + 1) * P],
                    ut[dc][:, g * QW + rs:(g + 1) * QW],
                    start=(dc == 0), stop=(dc == DC - 1),
                )
            score_ps[g][j] = ps

        def emit_post(g, j):
            # exp on the exact causal width; the partially-masked diagonal
            # 64-column wedge is zeroed multiplicatively after exp (8x less
            # DVE work than a 512-wide additive mask, and exp no longer
            # waits on the DVE)
            U = 8 * g + 8
            rs = rs_of(g, j)
            pt = ptpool.tile([P, QW], BF16, tag="pt")
            nc.scalar.activation(
                pt[:, rs:], score_ps[g][j][:, rs:],
                mybir.ActivationFunctionType.Exp, scale=SCALE)
            if j >= 8 * g:
                jj = j - 8 * g
                nc.vector.tensor_mul(
                    pt[:, rs:rs + 64], pt[:, rs:rs + 64],
                    mask_sb[:, jj * 64:(jj + 1) * 64])
            nc.tensor.matmul(
                l_ps[g][:, rs:], ones1[:, 0:1], pt[:, rs:],
                start=(j == 0), stop=(j == U - 1), skip_group_check=True,
            )
            pts[g][j] = pt

        def emit_unit_range(g, lo, hi):
            for j in range(lo, hi):
                emit_scores(g, j)
                if j > lo:
                    emit_post(g, j - 1)
            emit_post(g, hi - 1)

        def emit_linv(g):
            # PSUM -> SBUF bounce (DMA can't read PSUM), transpose l into
            # lanes (4 tiny column DMAs), then reciprocal on [128, 4]
            l_sb = linvtpool.tile([1, QW], F32, tag="lsb")
            nc.vector.tensor_copy(l_sb[:], l_ps[g][:])
            lt = linvtpool.tile([P, 8], F32, tag="linvt")
            for c in range(4):
                nc.scalar.dma_start(
                    out=lt[:, c:c + 1], in_=l_sb[0:1, c * P:(c + 1) * P])
            nc.vector.reciprocal(lt[:, 4:8], lt[:, 0:4])
            linv_t[g] = lt

        def emit_t(g):
            # tT[d, q'] = sum_k xr[k, d] p[k, q'], per d-chunk; unit j only
            # contributes to q' >= rs(j) (p is exactly 0 left of its band)
            U = 8 * g + 8
            tts = []
            for dc in range(DC):
                ps = ps_t.tile([P, QW], F32, tag="t")
                for j in range(U):
                    rs = rs_of(g, j)
                    nc.tensor.matmul(
                        ps[:, rs:],
                        xr_sb[j][:, dc * P:(dc + 1) * P],
                        pts[g][j][:, rs:],
                        start=(j == 0), stop=(j == U - 1),
                    )
                t = ttpool.tile([P, QW], BF16, tag="tt")
                nc.vector.tensor_copy(t[:], ps[:])
                tts.append(t)
            tt[g] = tts

        def emit_out(g):
            # out[q', o] = sum_d tT[d, q']^T wvT[d, o], then divide by l
            for qs in range(4):
                for oh in range(2):
                    ps = ps_o.tile([P, 512], F32, tag="o")
                    for dc in range(DC):
                        nc.tensor.matmul(
                            ps[:],
                            tt[g][dc][:, qs * P:(qs + 1) * P],
                            wv_sb[oh][:, dc * 512:(dc + 1) * 512],
                            start=(dc == 0), stop=(dc == DC - 1),
                        )
                    out_sb = outpool.tile([P, 512], F32, tag="out")
                    nc.scalar.mul(out_sb[:], ps[:],
                                  linv_t[g][:, 4 + qs:5 + qs])
                    r0 = g * 4 * P + qs * P
                    nc.sync.dma_start(
                        out=out[r0:r0 + P, oh * 512:(oh + 1) * 512],
                        in_=out_sb[:])

        l_ps[0] = ps_l.tile([1, QW], F32, tag="l", name="l0")
        l_ps[1] = ps_l.tile([1, QW], F32, tag="l", name="l1")
        emit_unit_range(0, 0, 8)      # g0 scores/exp/l
        emit_linv(0)
        emit_t(0)
        emit_unit_range(1, 0, 8)      # g1 first half overlaps g0 T/out
        emit_out(0)
        emit_unit_range(1, 8, 16)
        emit_linv(1)
        emit_t(1)
        emit_out(1)

    nc.compile()
    return nc


_PROGRAM = None


def _get_program():
    global _PROGRAM
    if _PROGRAM is None:
        _PROGRAM = _build_program()
    return _PROGRAM


# Set by kernel() after each run: BassKernelResults (exec_time_ns etc.)
last_results = None


def kernel(**inputs):
    global last_results
    _install_axon_profile_hook()

    import ml_dtypes

    x = np.asarray(inputs["x"], dtype=np.float32)
    wq = np.asarray(inputs["wq"], dtype=np.float32)
    wk = np.asarray(inputs["wk"], dtype=np.float32)
    wv = np.asarray(inputs["wv"], dtype=np.float32)

    # A = Wq^T Wk folds the Q and K projections into one: scores = (X A) X^T
    a = (wq.T.astype(np.float64) @ wk.astype(np.float64)).astype(np.float32)
    at = a.astype(ml_dtypes.bfloat16)                       # [d_in, d_in]
    wvt = np.ascontiguousarray(wv.T).astype(ml_dtypes.bfloat16)

    def tile4(w, nf):
        # [c*128+p, i*nf+f] -> [i*128+p, c*nf+f]  (SBUF tile layout)
        ni = w.shape[1] // nf
        return np.ascontiguousarray(
            w.reshape(DC, P, ni, nf).transpose(2, 1, 0, 3).reshape(
                ni * P, DC * nf))

    at_l = tile4(at, 256)                                   # [512, 2048]
    wvt_l = tile4(wvt, 512)                                 # [256, 4096]

    # own query rows per core half h: parity-h rows within each 1024-row group
    own_rows = {}
    for h in range(2):
        rows = []
        for g in range(G):
            rows.extend(range(1024 * g + h, 1024 * (g + 1), 2))
        own_rows[h] = np.array(rows, dtype=np.int64)

    # multiplicative 0/1 wedge masks [128, 8*64] bf16 for the partially
    # masked 64-column diagonal block of each within-group key-block unit
    masks = {}
    kl = np.arange(P)[:, None]
    for h in range(2):
        m = np.zeros((P, 8 * 64), dtype=np.float32)
        for t in range(8):
            qp = 64 * t + np.arange(64)[None, :]   # q' columns of the wedge
            krow = P * t + kl
            qrow = 2 * qp + h
            m[:, t * 64:(t + 1) * 64] = (krow <= qrow).astype(np.float32)
        masks[h] = m.astype(ml_dtypes.bfloat16)

    in_maps = []
    for c in range(N_CORES):
        b, h = divmod(c, 2)
        xb = x[b].astype(ml_dtypes.bfloat16)                # [S, D] rows
        xtb = np.ascontiguousarray(x[b].T).astype(ml_dtypes.bfloat16)
        in_maps.append({
            "xt": xtb,
            "xr": xb,
            "xq": tile4(np.ascontiguousarray(xtb[:, own_rows[h]]), 512),
            "at": at_l, "wvt": wvt_l,
            "mask": masks[h],
            "ones": np.ones((P, 512), dtype=ml_dtypes.bfloat16),
        })

    nc = _get_program()
    trace = bool(int(os.environ.get("KERNEL_TRACE", "0")))
    kwargs = {}
    if trace:
        kwargs["trace"] = True
        kwargs["trace_cores"] = list(range(N_CORES))
        tdir = os.environ.get("KERNEL_TRACE_DIR")
        if tdir:
            os.makedirs(tdir, exist_ok=True)
            kwargs["tmpdir"] = tdir
    res = run_bass_kernel_spmd(nc, in_maps, core_ids=list(range(N_CORES)),
                               **kwargs)
    last_results = res

    out = np.empty((B, S, D), dtype=np.float32)
    for c in range(N_CORES):
        b, h = divmod(c, 2)
        out[b, own_rows[h], :] = res.results[c]["out"]
    return out


# revision 23
# speedup vs baseline: 1.1383x; 1.1383x over previous
"""Trainium2 Bass kernel for single-head causal attention.

Problem: x[4, 2048, 1024] fp32; wq/wk/wv [1024, 1024] (torch layout [d_out, d_in]).
  q = x @ wq.T ; k = x @ wk.T ; v = x @ wv.T  (per batch)
  out = softmax(causal(q @ k.T) / 32) @ v

Sharding (8 cores): core c = 2*b + h owns batch b and half of its query rows.
Query rows are split between the two cores of a batch by parity *within* each
1024-row group so both cores see an identical causal work profile -> the SPMD
program is fully uniform; only data (inputs) differ per core. Cores are fully
independent (no collectives / cross-core traffic).

Key algebraic folding (removes the K and V projections entirely):
  scores = (X Wq^T)(X Wk^T)^T = X (Wq^T Wk) X^T
         = U X^T   with U := Xq A,  A := Wq^T Wk  (precomputed on HOST)
  out    = P V / l = (P X) Wv^T / l = T Wv^T / l  with T := P X
So the device only computes:  U (one [1024x1024x1024] projection per core),
scores against raw X^T, exp, T = P X, and out = T Wv^T. Per-core PE work is
~283K PE-columns vs ~490K for the direct form.

All matmuls are bf16 (1 cycle/row PE rate at any N, LDWEIGHTS hidden, half
the DMA bytes of fp32); PSUM accumulation stays fp32.

Per-core device program:
  phase 1: uT[di, q'] = A-chunks.T @ xqT   (SBUF-resident, like a projection)
           xt[dc] (= X^T, scores lhsT) and xr[kb] (= X rows, T lhsT) stream
           in by plain DMA (inputs, nothing to compute).
  phase 2: per group g (2 groups of 512 own-q columns, kb = 8g+8 key blocks):
      scoresT[k, q'] = sum_d xt.T @ uT  (PSUM, 8 d-chunk matmuls, exact
      causal width per key block; stale PSUM left of the band is killed by
      the additive mask); additive causal mask (DVE, bf16 -1e9 tiles);
      p = exp(scores/32) (ACT, PSUM->SBUF bf16); l[q'] += ones.T @ p (PE);
      T:  tT[d, q'] += xr-slices.T @ p   (per d-chunk, exact causal width)
      out[q', o] = sum_d tT-slices.T @ wvT-chunks, then ACT copy divides by
      l via per-partition scale AP (l transposed into lanes by 4 tiny column
      DMAs, then reciprocal on [128, 4]).
"""

import os
import sys
import types
from contextlib import ExitStack

for _p in ("/opt/trn_rl_repo", "/root/.axon_site/_ro/trn_rl_repo"):
    if os.path.isdir(_p) and _p not in sys.path:
        sys.path.insert(0, _p)

import numpy as np

import concourse.bacc as bacc
import concourse.mybir as mybir
import concourse.tile as tile
from concourse.bass_utils import run_bass_kernel_spmd

F32 = mybir.dt.float32
BF16 = mybir.dt.bfloat16

B, S, D = 4, 2048, 1024
P = 128
DC = D // P      # 8 d-chunks
SKB = S // P     # 16 key blocks
G = 2            # query groups per core
QW = 512         # query columns per group per core
SQ = G * QW      # 1024 own query rows per core
N_CORES = 8
SCALE = 1.0 / 32.0  # 1/sqrt(D)
NEG = -1e9


def _install_axon_profile_hook():
    """Provide antenv.axon_hooks (absent in this image) so trace=True works."""
    name = "antenv.axon_hooks"
    if name in sys.modules:
        return
    mod = types.ModuleType(name)
    _hook = [None]
    mod.set_axon_ntff_profile_hook = lambda h: _hook.__setitem__(0, h)
    mod.get_axon_ntff_profile_hook = lambda: _hook[0]
    sys.modules[name] = mod
    try:
        import antenv

        antenv.axon_hooks = mod
        from trn_agent_boot.trn_boot import _ntff_profile_via_ctypes

        mod.set_axon_ntff_profile_hook(
            _ntff_profile_via_ctypes("/opt/axon/libaxon_pjrt.so")
        )
    except Exception:
        pass


def _build_program():
    nc = bacc.Bacc("TRN2", target_bir_lowering=False, debug=False,
                   num_devices=N_CORES)

    xt = nc.dram_tensor("xt", [D, S], BF16, kind="ExternalInput").ap()
    xr = nc.dram_tensor("xr", [S, D], BF16, kind="ExternalInput").ap()
    # xq/at/wvt come pre-arranged on the host into exact SBUF tile layout
    # so every load is a contiguous 128x4KB DMA (128 descriptors), not a
    # strided rearrange (1024 small descriptors, ~10x slower issue).
    xq = nc.dram_tensor("xq", [2 * P, DC * 512], BF16,
                        kind="ExternalInput").ap()
    at = nc.dram_tensor("at", [4 * P, DC * 256], BF16,
                        kind="ExternalInput").ap()
    wvt = nc.dram_tensor("wvt", [2 * P, DC * 512], BF16,
                         kind="ExternalInput").ap()
    ones_in = nc.dram_tensor("ones", [P, 512], BF16, kind="ExternalInput").ap()
    mask = nc.dram_tensor("mask", [P, 8 * 64], BF16, kind="ExternalInput").ap()
    out = nc.dram_tensor("out", [SQ, D], F32, kind="ExternalOutput").ap()

    with tile.TileContext(nc, pool_alloc_mode="queue") as tc, ExitStack() as es:
        const = es.enter_context(tc.tile_pool(name="const", bufs=1))
        utpool = es.enter_context(tc.tile_pool(name="utpool", bufs=8))
        xtpool = es.enter_context(tc.tile_pool(name="xtpool", bufs=8))
        xrpool = es.enter_context(tc.tile_pool(name="xrpool", bufs=16))
        wvpool = es.enter_context(tc.tile_pool(name="wvpool", bufs=2))
        maskpool = es.enter_context(tc.tile_pool(name="maskpool", bufs=1))

        ones1 = const.tile([P, 512], BF16)
        nc.gpsimd.dma_start(out=ones1[:], in_=ones_in[:])

        p1 = ExitStack()
        # A o-quarter tiles [128, 8*256]: quarter i holds di-cols
        # [256i, 256(i+1)) for all 8 dj-chunks (lhsT slices for U).
        apool = p1.enter_context(tc.tile_pool(name="apool", bufs=4))
        xqpool = p1.enter_context(tc.tile_pool(name="xqpool", bufs=2))
        pp = p1.enter_context(tc.tile_pool(name="pp", bufs=2, space="PSUM"))

        def load_a(i, eng=None):
            w = apool.tile([P, DC * 256], BF16, tag="a")
            (eng or nc.sync).dma_start(
                out=w[:], in_=at[i * P:(i + 1) * P, :])
            return w

        def a_lhsT(a_sb, oc, dc):
            base = dc * 256 + (oc % 2) * P
            return a_sb[oc // 2][:, base:base + P]

        def load_xq(st, eng=None):
            t = xqpool.tile([P, DC * 512], BF16, tag="xq")
            (eng or nc.sync).dma_start(
                out=t[:], in_=xq[st * P:(st + 1) * P, :])
            return t

        # PE warmup: throwaway matmuls on the ones tile (lands in ~1us)
        # keep the PE busy through its p-state ramp while inputs stream in
        wu_ps = p1.enter_context(tc.tile_pool(name="wups", bufs=1,
                                              space="PSUM"))
        wups = wu_ps.tile([P, 512], F32, tag="wu")
        for i in range(12):
            nc.tensor.matmul(
                wups[:], ones1[:, 0:P], ones1[:, 0:512],
                start=(i == 0), stop=(i == 11), skip_group_check=True)

        # startup: first A quarter + first xq tile unblock the first U
        # accumulation group asap. Critical loads stay on the two fast HWDGE
        # queues (sync/scalar); the slow gpsimd SWDGE queue only carries bulk
        # tiles that aren't needed until scores/T time.
        a_sb = [load_a(0, eng=nc.sync)]
        xq_sb = [load_xq(0, eng=nc.scalar)]
        a_sb += [load_a(1, eng=nc.sync), load_a(2, eng=nc.scalar),
                 load_a(3, eng=nc.sync)]
        xq_sb.append(load_xq(1, eng=nc.scalar))

        # resident X^T chunk tiles (scores lhsT): one contiguous DMA each,
        # spread over all three queues (needed from scores-g0 time on)
        xt_engs = [nc.gpsimd, nc.gpsimd, nc.gpsimd, nc.gpsimd,
                   nc.sync, nc.sync, nc.scalar, nc.scalar]
        xt_sb = []
        for dc in range(DC):
            t = xtpool.tile([P, S], BF16, tag="xt", name=f"xt{dc}")
            xt_engs[dc].dma_start(out=t[:], in_=xt[dc * P:(dc + 1) * P, :])
            xt_sb.append(t)

        # resident X row-block tiles (T lhsT): one contiguous DMA each
        xr_engs = [nc.gpsimd] * 8 + [nc.sync] * 4 + [nc.scalar] * 4
        xr_sb = []
        for kb in range(SKB):
            t = xrpool.tile([P, D], BF16, tag="xr", name=f"xr{kb}")
            xr_engs[kb % 8 if kb < 8 else kb].dma_start(
                out=t[:], in_=xr[kb * P:(kb + 1) * P, :])
            xr_sb.append(t)

        # wv^T o-half tiles [128, 8*512] (moving operand for out = T Wv^T)
        def load_wv(i, eng=None):
            w = wvpool.tile([P, DC * 512], BF16, tag="wv")
            (eng or nc.sync).dma_start(
                out=w[:], in_=wvt[i * P:(i + 1) * P, :])
            return w

        wv_sb = [load_wv(0, eng=nc.sync), load_wv(1, eng=nc.scalar)]

        mask_sb = maskpool.tile([P, 8 * 64], BF16, tag="mask")
        nc.scalar.dma_start(out=mask_sb[:], in_=mask[:])

        # ---- phase 1: uT projection (resident) ----
        ut = [utpool.tile([P, SQ], BF16, tag="ut", name=f"ut{i}")
              for i in range(DC)]
        for st in range(2):
            for oc in range(DC):
                ps = pp.tile([P, 512], F32, tag="pp")
                for dc in range(DC):
                    nc.tensor.matmul(
                        ps[:],
                        a_lhsT(a_sb, oc, dc),
                        xq_sb[st][:, dc * 512:(dc + 1) * 512],
                        start=(dc == 0), stop=(dc == DC - 1),
                    )
                nc.vector.tensor_copy(
                    ut[oc][:, st * 512:(st + 1) * 512], ps[:])
        p1.close()

        # ---- phase 2: attention ----
        ptpool = es.enter_context(tc.tile_pool(name="ptpool", bufs=16))
        ttpool = es.enter_context(tc.tile_pool(name="ttpool", bufs=10))
        linvtpool = es.enter_context(tc.tile_pool(name="linvtpool", bufs=2))
        outpool = es.enter_context(tc.tile_pool(name="outpool", bufs=2))
        ps_s = es.enter_context(tc.tile_pool(name="ps_s", bufs=2, space="PSUM"))
        ps_l = es.enter_context(tc.tile_pool(name="ps_l", bufs=1, space="PSUM"))
        ps_t = es.enter_context(tc.tile_pool(name="ps_t", bufs=2, space="PSUM"))
        ps_o = es.enter_context(tc.tile_pool(name="ps_o", bufs=2, space="PSUM"))

        # Per-group state; group g has U = 8g+8 key-block units.
        l_ps = {}
        pts = {0: [None] * 8, 1: [None] * 16}
        score_ps = {0: [None] * 8, 1: [None] * 16}
        linv_t = {}
        tt = {}

        def rs_of(g, j):
            # key block j contributes only to q' >= 64*(j-8g); exact causal
            # width (bf16 runs full rate at any N). Everything downstream
            # (exp, wedge, l, T) reads only [rs:], so PSUM left of rs is
            # never touched.
            jj = j - 8 * g
            return 64 * jj if j >= 8 * g else 0

        def emit_scores(g, j):
            rs = rs_of(g, j)
            ps = ps_s.tile([P, QW], F32, tag="s")
            for dc in range(DC):
                nc.tensor.matmul(
                    ps[:, rs:],
                    xt_sb[dc][:, j * P:(j + 1) * P],
                    ut[dc][:, g * QW + rs:(g + 1) * QW],
                    start=(dc == 0), stop=(dc == DC - 1),
                )
            score_ps[g][j] = ps

        def emit_post(g, j):
            # exp on the exact causal width; the partially-masked diagonal
            # 64-column wedge is zeroed multiplicatively after exp (8x less
            # DVE work than a 512-wide additive mask, and exp no longer
            # waits on the DVE)
            U = 8 * g + 8
            rs = rs_of(g, j)
            pt = ptpool.tile([P, QW], BF16, tag="pt")
            nc.scalar.activation(
                pt[:, rs:], score_ps[g][j][:, rs:],
                mybir.ActivationFunctionType.Exp, scale=SCALE)
            if j >= 8 * g:
                jj = j - 8 * g
                nc.vector.tensor_mul(
                    pt[:, rs:rs + 64], pt[:, rs:rs + 64],
                    mask_sb[:, jj * 64:(jj + 1) * 64])
            nc.tensor.matmul(
                l_ps[g][:, rs:], ones1[:, 0:1], pt[:, rs:],
                start=(j == 0), stop=(j == U - 1), skip_group_check=True,
            )
            pts[g][j] = pt

        def emit_unit_range(g, lo, hi):
            for j in range(lo, hi):
                emit_scores(g, j)
                if j > lo:
                    emit_post(g, j - 1)
            emit_post(g, hi - 1)

        def emit_linv(g):
            # PSUM -> SBUF bounce (DMA can't read PSUM), transpose l into
            # lanes (4 tiny column DMAs), then reciprocal on [128, 4]
            l_sb = linvtpool.tile([1, QW], F32, tag="lsb")
            nc.vector.tensor_copy(l_sb[:], l_ps[g][:])
            lt = linvtpool.tile([P, 8], F32, tag="linvt")
            for c in range(4):
                nc.scalar.dma_start(
                    out=lt[:, c:c + 1], in_=l_sb[0:1, c * P:(c + 1) * P])
            nc.vector.reciprocal(lt[:, 4:8], lt[:, 0:4])
            linv_t[g] = lt

        def emit_t(g):
            # tT[d, q'] = sum_k xr[k, d] p[k, q'], per d-chunk; unit j only
            # contributes to q' >= rs(j) (p is exactly 0 left of its band)
            U = 8 * g + 8
            tts = []
            for dc in range(DC):
                ps = ps_t.tile([P, QW], F32, tag="t")
                for j in range(U):
                    rs = rs_of(g, j)
                    nc.tensor.matmul(
                        ps[:, rs:],
                        xr_sb[j][:, dc * P:(dc + 1) * P],
                        pts[g][j][:, rs:],
                        start=(j == 0), stop=(j == U - 1),
                    )
                t = ttpool.tile([P, QW], BF16, tag="tt")
                nc.vector.tensor_copy(t[:], ps[:])
                tts.append(t)
            tt[g] = tts

        def emit_out(g):
            # out[q', o] = sum_d tT[d, q']^T wvT[d, o], then divide by l
            for qs in range(4):
                for oh in range(2):
                    ps = ps_o.tile([P, 512], F32, tag="o")
                    for dc in range(DC):
                        nc.tensor.matmul(
                            ps[:],
                            tt[g][dc][:, qs * P:(qs + 1) * P],
                            wv_sb[oh][:, dc * 512:(dc + 1) * 512],
                            start=(dc == 0), stop=(dc == DC - 1),
                        )
                    out_sb = outpool.tile([P, 512], F32, tag="out")
                    nc.scalar.mul(out_sb[:], ps[:],
                                  linv_t[g][:, 4 + qs:5 + qs])
                    r0 = g * 4 * P + qs * P
                    nc.sync.dma_start(
                        out=out[r0:r0 + P, oh * 512:(oh + 1) * 512],
                        in_=out_sb[:])

        l_ps[0] = ps_l.tile([1, QW], F32, tag="l", name="l0")
        l_ps[1] = ps_l.tile([1, QW], F32, tag="l", name="l1")
        emit_unit_range(0, 0, 8)      # g0 scores/exp/l
        emit_linv(0)
        emit_t(0)
        emit_unit_range(1, 0, 8)      # g1 first half overlaps g0 T/out
        emit_out(0)
        emit_unit_range(1, 8, 16)
        emit_linv(1)
        emit_t(1)
        emit_out(1)

    nc.compile()
    return nc


_PROGRAM = None


def _get_program():
    global _PROGRAM
    if _PROGRAM is None:
        _PROGRAM = _build_program()
    return _PROGRAM


# Set by kernel() after each run: BassKernelResults (exec_time_ns etc.)
last_results = None


def kernel(**inputs):
    global last_results
    _install_axon_profile_hook()

    import ml_dtypes

    x = np.asarray(inputs["x"], dtype=np.float32)
    wq = np.asarray(inputs["wq"], dtype=np.float32)
    wk = np.asarray(inputs["wk"], dtype=np.float32)
    wv = np.asarray(inputs["wv"], dtype=np.float32)

    # A = Wq^T Wk folds the Q and K projections into one: scores = (X A) X^T
    a = (wq.T.astype(np.float64) @ wk.astype(np.float64)).astype(np.float32)
    at = a.astype(ml_dtypes.bfloat16)                       # [d_in, d_in]
    wvt = np.ascontiguousarray(wv.T).astype(ml_dtypes.bfloat16)

    def tile4(w, nf):
        # [c*128+p, i*nf+f] -> [i*128+p, c*nf+f]  (SBUF tile layout)
        ni = w.shape[1] // nf
        return np.ascontiguousarray(
            w.reshape(DC, P, ni, nf).transpose(2, 1, 0, 3).reshape(
                ni * P, DC * nf))

    at_l = tile4(at, 256)                                   # [512, 2048]
    wvt_l = tile4(wvt, 512)                                 # [256, 4096]

    # own query rows per core half h: parity-h rows within each 1024-row group
    own_rows = {}
    for h in range(2):
        rows = []
        for g in range(G):
            rows.extend(range(1024 * g + h, 1024 * (g + 1), 2))
        own_rows[h] = np.array(rows, dtype=np.int64)

    # multiplicative 0/1 wedge masks [128, 8*64] bf16 for the partially
    # masked 64-column diagonal block of each within-group key-block unit
    masks = {}
    kl = np.arange(P)[:, None]
    for h in range(2):
        m = np.zeros((P, 8 * 64), dtype=np.float32)
        for t in range(8):
            qp = 64 * t + np.arange(64)[None, :]   # q' columns of the wedge
            krow = P * t + kl
            qrow = 2 * qp + h
            m[:, t * 64:(t + 1) * 64] = (krow <= qrow).astype(np.float32)
        masks[h] = m.astype(ml_dtypes.bfloat16)

    in_maps = []
    for c in range(N_CORES):
        b, h = divmod(c, 2)
        xb = x[b].astype(ml_dtypes.bfloat16)                # [S, D] rows
        xtb = np.ascontiguousarray(x[b].T).astype(ml_dtypes.bfloat16)
        in_maps.append({
            "xt": xtb,
            "xr": xb,
            "xq": tile4(np.ascontiguousarray(xtb[:, own_rows[h]]), 512),
            "at": at_l, "wvt": wvt_l,
            "mask": masks[h],
            "ones": np.ones((P, 512), dtype=ml_dtypes.bfloat16),
        })

    nc = _get_program()
    trace = bool(int(os.environ.get("KERNEL_TRACE", "0")))
    kwargs = {}
    if trace:
        kwargs["trace"] = True
        kwargs["trace_cores"] = list(range(N_CORES))
        tdir = os.environ.get("KERNEL_TRACE_DIR")
        if tdir:
            os.makedirs(tdir, exist_ok=True)
            kwargs["tmpdir"] = tdir
    res = run_bass_kernel_spmd(nc, in_maps, core_ids=list(range(N_CORES)),
                               **kwargs)
    last_results = res

    out = np.empty((B, S, D), dtype=np.float32)
    for c in range(N_CORES):
        b, h = divmod(c, 2)
        out[b, own_rows[h], :] = res.results[c]["out"]
    return out
